# revision 16
# baseline (speedup 1.0000x reference)
"""Trainium2 Bass kernel for nn_LocalizationLoss (B=128, N=65536).

Data-parallel over 8 NeuronCores: core m takes batches [16m, 16(m+1)).
Each core streams its 50 MB shard once, computing per-partition partial
sums of every loss term with fused-accumulate instructions
(ScalarE activation(accum_out=...) for transcendentals,
VectorE scalar_tensor_tensor(accum_out=...) for products).
Host combines the 8x[128,*] partials in float64.

Loss decomposition (per element; p* from `output`, t* from `target`):
  ce_pres*BN  = -S[t0*ln(p0)] - S[ln(1-p0)] + S[t0*ln(1-p0)]
  ce_class    = -S[ln(1-q_c)] (c=0..2)  - S[g_c*ln(q_c)] + S[g_c*ln(1-q_c)]
                  where g_c = (t4==c)*t0
  Lx*BN       = S[(p1-t1)^2]
  Ly*BN       = S[(p2-t2)^2]
  Lwh*BN      = S[p3+t3] - 2*S[sqrt(p3*t3)],  sqrt(x) = exp(0.5*ln(x))
  loss = 5*Lx + 5*Ly + 10*Lwh + 0.5 + 0.5*ce_pres + ce_class

ln/exp/square all live in the `natural_log_exp_and_others` ACT table set,
so the scalar engine never pays a table switch after the first load.
"""

import sys
from contextlib import ExitStack

if "/opt/trn_rl_repo" not in sys.path:
    sys.path.insert(0, "/opt/trn_rl_repo")

import numpy as np

import concourse.bass as bass
import concourse.mybir as mybir
import concourse.tile as tile
from concourse.bass_utils import run_bass_kernel_spmd

F32 = mybir.dt.float32
AF = mybir.ActivationFunctionType
ALU = mybir.AluOpType

# --- tail patch: the kernel-tail Drain cannot encode 10+ sync waits in one
# instruction (walrus "Too many sync wait commands").  Emit one drain per
# busy proc lane, each carrying a single wait, then finish with plain
# drain + barriers (replicating TileContext._drain_and_barrier).
import re as _re

from concourse.tile import ScopedClock as _ScopedClock
from concourse.tile import VectorClock as _VectorClock


def _patched_drain_and_barrier(self, tick_clock, wait_clock):
    ticks = [int(x) for x in _re.findall(r"\d+", repr(tick_clock.global_clock))]
    for proc, tk in enumerate(ticks):
        if tk > 0:
            part = _VectorClock()
            part.require_at_least(proc, tk)
            d = self.nc.sync.drain()
            wait_clock.add_sem_waits(d.ins, _ScopedClock({None: part}))
    self.nc.sync.drain()
    self.nc.all_engine_barrier()
    assert self.sems is not None
    popped = self.nc._tile_sem_poison_stack.pop()
    assert popped is self._sem_poison
    self.nc.clear_and_free_semaphores(list(self.sems.allocated().values()))
    self.nc.all_engine_barrier()


tile.TileContext._drain_and_barrier = _patched_drain_and_barrier

B, N = 128, 65536
NCORES = 8
PB = B // NCORES          # batches per core
P = 128                   # SBUF partitions

NSA = 5                   # ACT accum slots/tile: s1, s4, s8, s9, s10
NSV = 5                   # DVE accum slots/tile: s2, s3, s5, s6, s7

_DMA_ENGINE = "gpsimd"    # "gpsimd" (SWDGE) or "sync" (HWDGE)


def _emit(ctx, tc, x_ap, y_ap, acc_a_ap, acc_v_ap, rpp, T, in_bufs, mid_bufs):
    """Emit the per-core program. x:[PB,N,7] y:[PB,N,5] DRAM APs."""
    nc = tc.nc
    NT = rpp // T
    s = P // PB  # 8 partition-groups per batch
    xin = x_ap.rearrange("b (s n) c -> (b s) n c", s=s)   # [128, rpp, 7]
    yin = y_ap.rearrange("b (s n) c -> (b s) n c", s=s)   # [128, rpp, 5]

    iop = ctx.enter_context(tc.tile_pool(name="inp", bufs=in_bufs))
    mid = ctx.enter_context(tc.tile_pool(name="mid", bufs=mid_bufs))
    one = ctx.enter_context(tc.tile_pool(name="one", bufs=1))

    acc_a = one.tile([P, NT * NSA], F32)
    acc_v = one.tile([P, NT * NSV], F32)
    # per-tile probe slots (never rewritten -> no WAW sem waits ever)
    vprobe = one.tile([P, 3 * NT], F32)
    aprobe = one.tile([P, NT], F32)
    gprobe = one.tile([P, 2 * NT], F32)

    ldma = nc.gpsimd if _DMA_ENGINE == "gpsimd" else nc.sync
    for t in range(NT):
        ot = iop.tile([P, T, 7], F32, tag="ot")
        tt = iop.tile([P, T, 5], F32, tag="tt")
        ldma.dma_start(ot[:], xin[:, t * T:(t + 1) * T, :])
        ldma.dma_start(tt[:], yin[:, t * T:(t + 1) * T, :])

        p0 = ot[:, :, 0]
        px = ot[:, :, 1]
        py = ot[:, :, 2]
        pw = ot[:, :, 3]
        q3 = ot[:, :, 4:7]
        t0 = tt[:, :, 0]
        tx = tt[:, :, 1]
        ty = tt[:, :, 2]
        tw = tt[:, :, 3]
        kk = tt[:, :, 4]

        A = mid.tile([P, T], F32, tag="A")
        Bb = mid.tile([P, T], F32, tag="Bb")
        L = mid.tile([P, T, 3], F32, tag="L")
        M = mid.tile([P, T, 3], F32, tag="M")
        G = mid.tile([P, T, 3], F32, tag="G")
        r = mid.tile([P, T], F32, tag="r")
        lnr = mid.tile([P, T], F32, tag="lnr")
        dx = mid.tile([P, T], F32, tag="dx")
        dy = mid.tile([P, T], F32, tag="dy")
        jW = mid.tile([P, T], F32, tag="jW")

        def aa(i):
            j = t * NSA + i
            return acc_a[:, j:j + 1]

        def av(i):
            j = t * NSV + i
            return acc_v[:, j:j + 1]

        # Every engine instruction can encode only ONE sync-wait command.
        # 1-element "probe" copies absorb one new semaphore observation
        # each (input-DMA sems, cross-engine producer sems) so that every
        # real op below needs at most one new wait.  Probe slots are
        # written once per kernel (per-tile columns) -> no WAW waits.
        # In-place outputs (A,Bb,L,M,lnr,dx,dy) avoid shared-junk WAW.

        # ---- vector engine ----
        nc.vector.tensor_copy(vprobe[:, 3 * t:3 * t + 1], ot[:, 0:1, 0])
        nc.vector.tensor_copy(vprobe[:, 3 * t + 1:3 * t + 2], tt[:, 0:1, 0])
        for c in range(3):
            nc.vector.scalar_tensor_tensor(G[:, :, c], kk, float(c), t0,
                                           ALU.is_equal, ALU.mult)
        # reads the slice the LAST G writer produced, so the wait tick
        # covers all three G writers (engine retires in order)
        nc.vector.tensor_copy(vprobe[:, 3 * t + 2:3 * t + 3], G[:, 0:1, 2])
        nc.vector.scalar_tensor_tensor(r[:], pw, 0.0, tw,
                                       ALU.bypass, ALU.mult)
        nc.vector.scalar_tensor_tensor(dx[:], px, 0.0, tx,
                                       ALU.bypass, ALU.subtract)
        nc.vector.scalar_tensor_tensor(dy[:], py, 0.0, ty,
                                       ALU.bypass, ALU.subtract)

        # ---- scalar engine (all natural_log_exp table set) ----
        nc.scalar.copy(aprobe[:, t:t + 1], ot[:, 0:1, 0])
        nc.scalar.activation(A[:], p0, AF.Ln)
        nc.scalar.activation(Bb[:], p0, AF.Ln, scale=-1.0, bias=1.0,
                             accum_out=aa(0))                       # s1
        nc.scalar.activation(L[:], q3, AF.Ln)
        nc.scalar.activation(M[:], q3, AF.Ln, scale=-1.0, bias=1.0,
                             accum_out=aa(1))                       # s4
        nc.scalar.activation(lnr[:], r[:], AF.Ln)
        nc.scalar.activation(lnr[:], lnr[:], AF.Exp, scale=0.5,
                             accum_out=aa(2))                       # s8
        nc.scalar.activation(dx[:], dx[:], AF.Square,
                             accum_out=aa(3))                       # s9
        nc.scalar.activation(dy[:], dy[:], AF.Square,
                             accum_out=aa(4))                       # s10

        # ---- vector engine fused mult+accum ----
        nc.vector.scalar_tensor_tensor(A[:], A[:], 0.0, t0,
                                       ALU.bypass, ALU.mult, accum_out=av(0))
        nc.vector.scalar_tensor_tensor(Bb[:], Bb[:], 0.0, t0,
                                       ALU.bypass, ALU.mult, accum_out=av(1))
        nc.vector.scalar_tensor_tensor(L[:], G[:], 0.0, L[:],
                                       ALU.bypass, ALU.mult, accum_out=av(2))
        nc.vector.scalar_tensor_tensor(M[:], G[:], 0.0, M[:],
                                       ALU.bypass, ALU.mult, accum_out=av(3))
        nc.vector.scalar_tensor_tensor(jW[:], pw, 0.0, tw,
                                       ALU.bypass, ALU.add, accum_out=av(4))

        # ---- gpsimd probes: let the PL engine (which issues the input
        # DMA triggers) observe each compute engine's LAST reader of this
        # tile's inputs, so the reload trigger for buffer-slot reuse needs
        # only its own queue semaphore.
        # jW <- last DVE reader (sttW); acc_a slot 1 <- last ACT ot-reader
        # (the M pass).
        nc.gpsimd.tensor_copy(gprobe[:, 2 * t:2 * t + 1], jW[:, 0:1])
        nc.gpsimd.tensor_copy(gprobe[:, 2 * t + 1:2 * t + 2],
                              acc_a[:, t * NSA + 1:t * NSA + 2])

    nc.sync.dma_start(acc_a_ap[:, :], acc_a[:])
    nc.sync.dma_start(acc_v_ap[:, :], acc_v[:])


def build_program(pb=PB, n=N, T=512, in_bufs=3, mid_bufs=2):
    rows = pb * n
    rpp = rows // P
    NT = rpp // T
    assert rpp * P == rows and NT * T == rpp and n % rpp == 0

    nc = bass.Bass("TRN2", target_bir_lowering=False, debug=False)
    x = nc.dram_tensor("x", [pb, n, 7], F32, kind="ExternalInput")
    y = nc.dram_tensor("y", [pb, n, 5], F32, kind="ExternalInput")
    acc_a_d = nc.dram_tensor("acc_a", [P, NT * NSA], F32, kind="ExternalOutput")
    acc_v_d = nc.dram_tensor("acc_v", [P, NT * NSV], F32, kind="ExternalOutput")

    with tile.TileContext(nc) as tc:
        with ExitStack() as ctx:
            _emit(ctx, tc, x.ap(), y.ap(), acc_a_d.ap(), acc_v_d.ap(),
                  rpp, T, in_bufs, mid_bufs)
    return nc


def combine(acc_a_list, acc_v_list, n_elems):
    """Host-side float64 reduction of per-core partials -> scalar loss."""
    sa = np.zeros(NSA, dtype=np.float64)
    sv = np.zeros(NSV, dtype=np.float64)
    for a in acc_a_list:
        sa += a.astype(np.float64).reshape(P, -1, NSA).sum(axis=(0, 1))
    for v in acc_v_list:
        sv += v.astype(np.float64).reshape(P, -1, NSV).sum(axis=(0, 1))
    s1, s4, s8, s9, s10 = sa
    s2, s3, s5, s6, s7 = sv
    ce_pres = (-s2 - s1 + s3) / n_elems
    ce_class = -s4 - s5 + s6
    lx = s9 / n_elems
    ly = s10 / n_elems
    lwh = (s7 - 2.0 * s8) / n_elems
    loss = 5.0 * lx + 5.0 * ly + 10.0 * lwh + 0.5 + 0.5 * ce_pres + ce_class
    return np.float32(loss)


_CACHE = {}


def _get_nc(T=512, in_bufs=3, mid_bufs=2):
    key = (T, in_bufs, mid_bufs)
    if key not in _CACHE:
        _CACHE[key] = build_program(T=T, in_bufs=in_bufs, mid_bufs=mid_bufs)
    return _CACHE[key]


def kernel(output, target, _trace=False, _T=512, _in_bufs=3, _mid_bufs=2):
    assert output.shape == (B, N, 7) and target.shape == (B, N, 5)
    nc = _get_nc(_T, _in_bufs, _mid_bufs)
    in_maps = [
        {
            "x": np.ascontiguousarray(output[m * PB:(m + 1) * PB]),
            "y": np.ascontiguousarray(target[m * PB:(m + 1) * PB]),
        }
        for m in range(NCORES)
    ]
    res = run_bass_kernel_spmd(nc, in_maps, list(range(NCORES)), trace=_trace)
    loss = combine(
        [r["acc_a"] for r in res.results],
        [r["acc_v"] for r in res.results],
        float(B) * float(N),
    )
    if _trace:
        return loss, res
    return loss
